# revision 1
# baseline (speedup 1.0000x reference)
"""EntityGuidedCrossAttention TRN2 kernel (8 NeuronCores, data-parallel over classes).

Math restructure (exact): labels are contiguous per class, so attention is
block-diagonal.  With folded weights (host-side, weights-only algebra):
    Wqk = Wq^T Wk,  bqk = bq Wk          ->  Qk = ent @ Wqk + bqk
    Wvo = Wv^T Wo^T, bvo = bv Wo^T + bo  ->  OUT = pooled @ Wvo + bvo
    score[c,k] = Qk[c] . sup[c*K+k] / sqrt(D)   (bk is softmax-shift-invariant)
    pooled[c]  = sum_k softmax_w[c,k] * sup[c*K+k]
    res        = sup + OUT[class(row)]

Everything is bf16 in HBM/SBUF (f32 accumulation in PSUM / DVE accumulator):
in+out HBM traffic is ~21 MB/core vs 52.7 MB for the unfolded f32 version.

Device pipeline per core (64 classes / 4096 rows, 32 row-tiles of 128):
  A:   Qk = entT.T @ Wqk + bqk                  (PE bf16, 8 chunks, 1024-free)
  B:   per tile: qkb = onehot.T @ Qk (PE, one 1024-free mm) -> ACT drains
       PSUM to a bf16 SBUF ring (deep PE run-ahead; PE never waits on DVE);
       score = rowsum(sup*qkb/32) (DVE scalar_tensor_tensor, accum_out)
  C:   per 8-tile group: PE-transpose scores, softmax (reduce_max w/ negate,
       exp + renorm-scale on ACT), PE-transpose back, DVE-scatter the weight
       pairs into the padded lhsT bank w_all (tile t's cols at 66t, 66t+1)
  D:   pooled += w_all_t.T @ sup_t  (PE chained accumulation; the softmax
       weights ARE the lhsT -- no w*sup elementwise pass, no indt matrix)
  E:   OUT = pooledT.T @ Wvo + bvo              (PE, 8 chunks)
  F:   ob = onehot.T @ OUT (PE) -> ACT drains to bf16; res = sup + ob
       (DVE bf16 add, 2x mode); DMA out (bf16), host upcasts to f32
"""

import numpy as np

N_CLASSES = 512
K_SHOTS = 64
D = 1024
NK = N_CLASSES * K_SHOTS
N_CORES = 8
C_LOC = N_CLASSES // N_CORES          # 64 classes per core
R_LOC = NK // N_CORES                 # 4096 support rows per core
P = 128
TILES = R_LOC // P                    # 32 row-tiles of 128
DCH = D // P                          # 8 contraction chunks
GSZ = 8                               # tiles per softmax group
GROUPS = TILES // GSZ                 # 4
CPT = P // K_SHOTS                    # 2 classes per tile
INV_SQRT_D = 1.0 / float(np.sqrt(D))

_NC_CACHE = None


def _build_nc():
    import concourse.bacc as bacc
    import concourse.tile as tile
    import concourse.mybir as mybir
    from concourse.masks import make_identity

    f32 = mybir.dt.float32
    bf16 = mybir.dt.bfloat16
    AX = mybir.AxisListType.X
    ADD = mybir.AluOpType.add
    MUL = mybir.AluOpType.mult
    EXP = mybir.ActivationFunctionType.Exp
    CPY = mybir.ActivationFunctionType.Copy

    nc = bacc.Bacc("TRN2", target_bir_lowering=False, debug=False,
                   num_devices=N_CORES)

    sup_d = nc.dram_tensor("sup", [R_LOC, D], bf16, kind="ExternalInput").ap()
    entt_d = nc.dram_tensor("entt", [D, C_LOC], bf16, kind="ExternalInput").ap()
    ind_d = nc.dram_tensor("ind", [C_LOC, R_LOC], bf16, kind="ExternalInput").ap()
    wqk_d = nc.dram_tensor("wqk", [D, D], bf16, kind="ExternalInput").ap()
    wvo_d = nc.dram_tensor("wvo", [D, D], bf16, kind="ExternalInput").ap()
    bqk_d = nc.dram_tensor("bqk", [1, D], bf16, kind="ExternalInput").ap()
    bvo_d = nc.dram_tensor("bvo", [1, D], bf16, kind="ExternalInput").ap()
    res_d = nc.dram_tensor("res", [R_LOC, D], bf16, kind="ExternalOutput").ap()

    with tile.TileContext(nc) as tc:
        with tc.tile_pool(name="const", bufs=1) as const:
            idf = const.tile([P, P], f32)
            make_identity(nc, idf)
            idb = const.tile([P, P], bf16)
            nc.scalar.copy(out=idb, in_=idf)
            ones_b = const.tile([1, C_LOC], bf16)
            nc.vector.memset(ones_b, 1.0)
            ones_col = const.tile([P, 1], bf16)
            nc.vector.memset(ones_col, 1.0)

            entt_sb = const.tile([P, DCH * C_LOC], bf16)
            ind_sb = const.tile([C_LOC, R_LOC], bf16)
            wqk_sb = const.tile([P, DCH * D], bf16)
            wvo_sb = const.tile([P, DCH * D], bf16)
            bqk_sb = const.tile([1, D], bf16)
            bvo_sb = const.tile([1, D], bf16)
            qk_sb = const.tile([C_LOC, D], bf16)
            out_sb = const.tile([C_LOC, D], bf16)
            pooled_sb = const.tile([C_LOC, D], bf16)
            pooledt_sb = const.tile([P, DCH * C_LOC], bf16)
            sup_all = const.tile([P, TILES * D], bf16)
            # padded softmax-weight lhsT bank: tile t's two columns live at
            # flat offsets 66t, 66t+1; its lhsT slice is [64t, 64t+64)
            w_all = const.tile([P, 66 * TILES], bf16)
            nc.vector.memset(w_all, 0.0)

            # ---------------- input DMAs (all bf16, no staging) ------------
            nc.sync.dma_start(out=bqk_sb, in_=bqk_d)
            nc.sync.dma_start(out=bvo_sb, in_=bvo_d)
            nc.sync.dma_start(
                out=entt_sb.rearrange("p (ch c) -> p ch c", ch=DCH),
                in_=entt_d.rearrange("(ch p) c -> p ch c", p=P),
            )
            nc.sync.dma_start(out=ind_sb, in_=ind_d)
            wqk_v = wqk_sb.rearrange("p (ch d) -> p ch d", ch=DCH)
            wqkd_v = wqk_d.rearrange("(ch p) d -> p ch d", p=P)
            for h in range(4):
                nc.sync.dma_start(out=wqk_v[:, 2 * h:2 * h + 2, :],
                                  in_=wqkd_v[:, 2 * h:2 * h + 2, :])
            # sup in 8 chunks of 4 tiles (pipelines with phase B)
            sup_v = sup_all.rearrange("p (t d) -> p t d", d=D)
            supd_v = sup_d.rearrange("(t p) d -> p t d", p=P)
            for k in range(8):
                nc.sync.dma_start(out=sup_v[:, 4 * k:4 * k + 4, :],
                                  in_=supd_v[:, 4 * k:4 * k + 4, :])
            nc.sync.dma_start(
                out=wvo_sb.rearrange("p (ch d) -> p ch d", ch=DCH),
                in_=wvo_d.rearrange("(ch p) d -> p ch d", p=P),
            )

            # ---------------- PE warmup ------------------------------------
            # The PE p-state only ramps to full clock after ~3us of
            # continuous work; the first real matmul otherwise waits ~10us
            # for the wqk DMA with the clock cold.  Chew on the identity
            # during the DMA window to arrive at phase A hot.
            with (
                nc.named_scope("warmup"),
                tc.tile_pool(name="psW", bufs=2, space="PSUM") as psW,
            ):
                for _ in range(24):
                    w_ps = psW.tile([P, P], f32, tag="w", bufs=2)
                    nc.tensor.transpose(w_ps, idf, idf)

            # ---------------- Phase A: Qk = entT.T @ Wqk + bqk -------------
            with (
                nc.named_scope("phaseA"),
                tc.tile_pool(name="psA", bufs=1, space="PSUM") as psA,
            ):
                q_ps = psA.tile([C_LOC, D], f32)
                for ch in range(DCH):
                    for nh in range(2):
                        nc.tensor.matmul(
                            q_ps[:, nh * 512:(nh + 1) * 512],
                            entt_sb[:, ch * C_LOC:(ch + 1) * C_LOC],
                            wqk_sb[:, ch * D + nh * 512:ch * D + (nh + 1) * 512],
                            start=(ch == 0), stop=False,
                        )
                for nh in range(2):
                    nc.tensor.matmul(
                        q_ps[:, nh * 512:(nh + 1) * 512],
                        ones_b, bqk_sb[0:1, nh * 512:(nh + 1) * 512],
                        start=False, stop=True,
                    )
                nc.scalar.copy(out=qk_sb, in_=q_ps)

            # ------------- Phases B/C/D: scores, softmax, pooled -----------
            with (
                nc.named_scope("phaseBCD"),
                tc.tile_pool(name="sbB", bufs=2) as sbB,
                tc.tile_pool(name="psB", bufs=2, space="PSUM") as psB,
                tc.tile_pool(name="psP", bufs=1, space="PSUM") as psP,
            ):
                pooled_ps = psP.tile([C_LOC, D], f32)
                r_ps = psP.tile([C_LOC, 1], f32, name="r_ps")
                prod = sbB.tile([P, D], bf16, tag="prod", bufs=1)
                prod_g = sbB.tile([P, D], bf16, tag="prod_g", bufs=1)
                for g in range(GROUPS):
                    s8 = sbB.tile([P, GSZ], f32, tag="s8", bufs=2)
                    for j in range(GSZ):
                        t = g * GSZ + j
                        qkb = psB.tile([P, D], f32, tag="qkb", bufs=2)
                        for nh in range(2):
                            nc.tensor.matmul(
                                qkb[:, nh * 512:(nh + 1) * 512],
                                ind_sb[:, t * P:(t + 1) * P],
                                qk_sb[:, nh * 512:(nh + 1) * 512],
                                start=True, stop=True,
                            )
                        # ACT drains PSUM fast so the PE never waits on DVE
                        qkb_sb = sbB.tile([P, D], bf16, tag="qkb_sb", bufs=4)
                        nc.scalar.copy(out=qkb_sb, in_=qkb)
                        eng = nc.vector
                        eng.scalar_tensor_tensor(
                            out=prod,
                            in0=sup_all[:, t * D:(t + 1) * D],
                            scalar=INV_SQRT_D,
                            in1=qkb_sb,
                            op0=MUL,
                            op1=MUL,
                            accum_out=s8[:, j:j + 1],
                        )
                    # unnormalized softmax: scores are O(+-6) so exp without
                    # max-subtraction is safe; normalization is deferred to a
                    # per-class 1/r scale when pooled leaves PSUM (r is
                    # accumulated from the SAME bf16 e values, so the
                    # normalization is exact w.r.t. quantized weights)
                    e8 = sbB.tile([P, GSZ], bf16, tag="e8", bufs=2)
                    nc.scalar.activation(out=e8, in_=s8, func=EXP)
                    # scatter tile t's weight pair to flat cols 66t, 66t+1
                    b0 = 66 * GSZ * g
                    nc.vector.tensor_copy(
                        out=w_all[0:K_SHOTS, b0:b0 + 66 * GSZ:66],
                        in_=e8[0:K_SHOTS, :])
                    nc.vector.tensor_copy(
                        out=w_all[K_SHOTS:P,
                                  b0 + 1:b0 + 66 * (GSZ - 1) + 2:66],
                        in_=e8[K_SHOTS:P, :])
                    # D: pooled += w_all_t.T @ sup_t, r += w_all_t.T @ ones
                    for j in range(GSZ):
                        t = g * GSZ + j
                        for nh in range(2):
                            nc.tensor.matmul(
                                pooled_ps[:, nh * 512:(nh + 1) * 512],
                                w_all[:, 64 * t:64 * t + 64],
                                sup_all[:, t * D + nh * 512:
                                        t * D + (nh + 1) * 512],
                                start=(t == 0), stop=(t == TILES - 1),
                            )
                        nc.tensor.matmul(
                            r_ps,
                            w_all[:, 64 * t:64 * t + 64],
                            ones_col,
                            start=(t == 0), stop=(t == TILES - 1),
                        )
                ri_sb = sbB.tile([C_LOC, 1], f32, tag="ri_sb", bufs=1)
                nc.vector.reciprocal(ri_sb, r_ps)
                nc.scalar.activation(out=pooled_sb, in_=pooled_ps,
                                     func=CPY, scale=ri_sb[:, 0:1])

            # ---------------- Phase E: OUT = pooledT.T @ Wvo + bvo ---------
            with (
                nc.named_scope("phaseE"),
                tc.tile_pool(name="psE", bufs=2, space="PSUM") as psE,
            ):
                for ch in range(DCH):
                    tp_ps = psE.tile([P, C_LOC], bf16, tag="tp")
                    nc.tensor.transpose(
                        tp_ps, pooled_sb[:, ch * P:(ch + 1) * P],
                        idb[0:C_LOC, 0:C_LOC],
                    )
                    nc.scalar.copy(
                        out=pooledt_sb[:, ch * C_LOC:(ch + 1) * C_LOC],
                        in_=tp_ps,
                    )
                o_ps = psE.tile([C_LOC, D], f32, tag="proj")
                for ch in range(DCH):
                    for nh in range(2):
                        nc.tensor.matmul(
                            o_ps[:, nh * 512:(nh + 1) * 512],
                            pooledt_sb[:, ch * C_LOC:(ch + 1) * C_LOC],
                            wvo_sb[:, ch * D + nh * 512:ch * D + (nh + 1) * 512],
                            start=(ch == 0), stop=False,
                        )
                for nh in range(2):
                    nc.tensor.matmul(
                        o_ps[:, nh * 512:(nh + 1) * 512],
                        ones_b, bvo_sb[0:1, nh * 512:(nh + 1) * 512],
                        start=False, stop=True,
                    )
                nc.scalar.copy(out=out_sb, in_=o_ps)

            # ---------------- Phase F: res = sup + OUT[class(row)] ---------
            res_v = res_d.rearrange("(t p) d -> p t d", p=P)
            with (
                nc.named_scope("phaseF"),
                tc.tile_pool(name="sbF", bufs=2) as sbF,
                tc.tile_pool(name="psF", bufs=2, space="PSUM") as psF,
            ):
                for t in range(TILES):
                    ob = psF.tile([P, D], f32, tag="ob", bufs=4)
                    for nh in range(2):
                        nc.tensor.matmul(
                            ob[:, nh * 512:(nh + 1) * 512],
                            ind_sb[:, t * P:(t + 1) * P],
                            out_sb[:, nh * 512:(nh + 1) * 512],
                            start=True, stop=True,
                        )
                    ob_sb = sbF.tile([P, D], bf16, tag="ob_sb", bufs=3)
                    nc.scalar.copy(out=ob_sb, in_=ob)
                    nc.vector.tensor_tensor(
                        out=sup_all[:, t * D:(t + 1) * D],
                        in0=sup_all[:, t * D:(t + 1) * D],
                        in1=ob_sb,
                        op=ADD,
                    )
                    if t % 4 == 3:
                        nc.sync.dma_start(
                            out=res_v[:, t - 3:t + 1, :],
                            in_=sup_v[:, t - 3:t + 1, :],
                        )

    nc.compile()
    return nc


def _get_nc():
    global _NC_CACHE
    if _NC_CACHE is None:
        _NC_CACHE = _build_nc()
    return _NC_CACHE


def _prep_in_maps(support_features, entity_vectors, support_labels,
                  Wq, bq, Wk, bk, Wv, bv, Wo, bo):
    from ml_dtypes import bfloat16

    sup = np.asarray(support_features, dtype=np.float32)
    ent = np.asarray(entity_vectors, dtype=np.float32)
    labels = np.asarray(support_labels, dtype=np.int32)
    wq = np.asarray(Wq, dtype=np.float32)
    wk = np.asarray(Wk, dtype=np.float32)
    wv = np.asarray(Wv, dtype=np.float32)
    wo = np.asarray(Wo, dtype=np.float32)
    bq_ = np.asarray(bq, dtype=np.float32).reshape(1, D)
    bv_ = np.asarray(bv, dtype=np.float32).reshape(1, D)
    bo_ = np.asarray(bo, dtype=np.float32).reshape(1, D)
    # bk is dropped: it adds a per-class constant to each softmax row.

    # weights-only folding (reparameterization; activation math is on-device)
    wqk = np.ascontiguousarray(wq.T @ wk).astype(bfloat16)
    wvo = np.ascontiguousarray(wv.T @ wo.T).astype(bfloat16)
    bqk = (bq_ @ wk).astype(bfloat16)
    bvo = (bv_ @ wo.T + bo_).astype(bfloat16)

    expected = np.arange(NK, dtype=np.int32) // K_SHOTS
    assert np.array_equal(labels, expected), (
        "kernel assumes exactly K_SHOTS contiguous samples per class "
        "(labels == arange(NK)//K_SHOTS)"
    )

    sup_bf = sup.astype(bfloat16)
    in_maps = []
    for c in range(N_CORES):
        lab_loc = labels[c * R_LOC:(c + 1) * R_LOC] - c * C_LOC
        ind = (lab_loc[None, :] ==
               np.arange(C_LOC, dtype=np.int32)[:, None]).astype(bfloat16)
        in_maps.append({
            "sup": np.ascontiguousarray(sup_bf[c * R_LOC:(c + 1) * R_LOC]),
            "entt": np.ascontiguousarray(
                ent[c * C_LOC:(c + 1) * C_LOC].T).astype(bfloat16),
            "ind": np.ascontiguousarray(ind),
            "wqk": wqk, "wvo": wvo, "bqk": bqk, "bvo": bvo,
        })
    return in_maps


def _run(in_maps, **kwargs):
    from concourse.bass_utils import run_bass_kernel_spmd
    nc = _get_nc()
    return run_bass_kernel_spmd(nc, in_maps, core_ids=list(range(N_CORES)),
                                **kwargs)


def kernel(support_features, entity_vectors, support_labels,
           Wq, bq, Wk, bk, Wv, bv, Wo, bo):
    in_maps = _prep_in_maps(support_features, entity_vectors, support_labels,
                            Wq, bq, Wk, bk, Wv, bv, Wo, bo)
    r = _run(in_maps)
    return np.concatenate(
        [np.asarray(r.results[c]["res"], dtype=np.float32)
         for c in range(N_CORES)], axis=0)

